# revision 1
# baseline (speedup 1.0000x reference)
"""Trainium2 Bass kernel for multi-head attention (dense transformer block).

Reference computation (per batch element):
    qkv = x @ w_qkv                      # [N, 3C]
    q, k, v = split heads (H=12, HD=64); q *= HD**-0.5
    out = softmax(q k^T) v               # full [N, N] scores
    out = merge_heads(out) @ w_proj + b_proj

Distribution: pure data parallel over the batch dim — B=8 batch elements,
8 NeuronCores, one element per core.  Weights are replicated.  No
collectives are needed; each core computes its full [2048, 768] output.

Per-core compute strategy (all matmuls bf16, fp32 PSUM accumulation):
  * x is cast f32->bf16 by a SWDGE DMA into a DRAM scratch, then DMA-xbar
    transposed into SBUF as xT [768, 2048] (feature-on-partition layout).
  * qkT = w_qk^T @ x^T -> [1536, 2048]: q/k for a head PAIR live in one
    128-partition tile (head A on partitions 0-63, head B on 64-127), so
    the K=64 score matmuls auto-pack as 64x128 row tiles of the PE array.
  * v = x @ w_v -> [2048, 768] natural layout (keys on partitions), which
    is exactly the lhsT needed for the attnV matmuls.
  * scoresT[m, n] = kT^T qT per head: keys on partitions, queries on the
    free dim.  exp() runs on ScalarE straight out of PSUM at FD=1024 (a
    head pair's [128, 2x512] chunk per instruction), with the 1/8
    softmax scale folded into the activation's free affine.  No max
    subtraction: scaled scores are ~N(0,1) so exp never overflows.
  * attnV: outT_h = v_h^T @ A_T^h accumulated over the 16 key tiles.  The
    two heads of a pair auto-pack as 128x64 column tiles (head A ->
    PSUM partitions 0-63, head B -> 64-127) sharing one PSUM bank.
  * softmax denominators: ones^T @ A_T matmuls, four heads (a "quad")
    packed as 128x32 column tiles into one PSUM bank.
  * normalization (divide by denominators) is applied at the attnV
    PSUM->SBUF eviction: reciprocal on DVE, broadcast across partitions
    via a DMA bounce, one tensor_tensor multiply.
  * final = outT^T @ w_proj with b_proj preloaded into PSUM by a K=1
    ones-matmul, evicted f32 and DMA'd out.
"""

import os

import numpy as np

import concourse.bass as bass
import concourse.mybir as mybir
from concourse import bacc, bass_utils
from concourse.tile import TileContext

F32 = mybir.dt.float32
BF16 = mybir.dt.bfloat16
AF = mybir.ActivationFunctionType

B, N, C = 8, 2048, 768
H, HD = 12, 64
SCALE = HD ** -0.5  # folded into the exp activation
P = 128
NT = N // P          # 16 token tiles
CT = C // P          # 6 feature tiles
NCHUNK = 4           # query chunks of 512
QW = N // NCHUNK     # 512


def build_nc() -> bass.Bass:
    nc = bacc.Bacc(None)
    x = nc.declare_dram_parameter("x", [N, C], F32, isOutput=False)
    w_qkv = nc.declare_dram_parameter("w_qkv", [C, 3 * C], F32, isOutput=False)
    w_proj = nc.declare_dram_parameter("w_proj", [C, C], F32, isOutput=False)
    b_proj = nc.declare_dram_parameter("b_proj", [C], F32, isOutput=False)
    out = nc.declare_dram_parameter("out", [N, C], F32, isOutput=True)

    with TileContext(nc) as tc:
        with (
            tc.tile_pool(name="const", bufs=1) as cpool,
            tc.tile_pool(name="dram", bufs=1, space="DRAM") as dpool,
            tc.tile_pool(name="rdram", bufs=2, space="DRAM") as rdpool,
            tc.tile_pool(name="at", bufs=6) as at_pool,
            tc.tile_pool(name="recip", bufs=2) as recip_pool,
            tc.tile_pool(name="rbc", bufs=2) as rbc_pool,
            tc.tile_pool(name="fin", bufs=2) as fin_pool,
            tc.tile_pool(name="psc", bufs=2, space="PSUM") as psum_sc,
            tc.tile_pool(name="pav", bufs=2, space="PSUM") as psum_av,
            tc.tile_pool(name="psum1", bufs=1, space="PSUM") as psum_sums,
            tc.tile_pool(name="pproj", bufs=1, space="PSUM") as psum_proj,
        ):
            # ---- persistent SBUF tensors -------------------------------
            w_qkv_sb = cpool.tile([P, CT, 3 * C], BF16, tag="wqkv")
            wproj_sb = cpool.tile([P, CT, C], BF16, tag="wproj")
            b_bc = cpool.tile([P, C], F32, tag="bias")  # bias bcast to 128 rows
            ones128 = cpool.tile([P, 1], BF16, tag="ones128")
            xT = cpool.tile([P, CT, N], BF16, tag="xT")
            qkT = cpool.tile([P, 12, N], BF16, tag="qkT")  # 12 = q(6 pairs)+k(6)
            v4 = cpool.tile([P, NT, C], BF16, tag="v4")
            outT = cpool.tile([P, CT, N], BF16, tag="outT")

            # ---- phase 0: load + cast + transpose ----------------------
            # interleave the x-cast chain with per-chunk w_qkv casts on the
            # SWDGE queue so the first qkT matmul's inputs (xT ct0 + w ct0)
            # are both ready within a few us; w_proj/bias load last.
            nc.any.memset(ones128[:], 1.0)
            nc.gpsimd.dma_start(
                out=w_qkv_sb[:], in_=w_qkv.rearrange("(o p) j -> p o j", p=P)
            )
            nc.gpsimd.dma_start(
                out=wproj_sb[:], in_=w_proj.rearrange("(o p) j -> p o j", p=P)
            )
            nc.sync.dma_start(
                out=b_bc[:], in_=b_proj[None, :].to_broadcast((P, C))
            )
            x_bf = dpool.tile([N, C], BF16)
            for ct in range(CT):
                csl = slice(ct * P, (ct + 1) * P)
                # per-column-chunk cast so each transpose starts early
                nc.gpsimd.dma_start(out=x_bf[:, csl], in_=x[:, csl])
                nc.sync.dma_start_transpose(xT[:, ct, :], x_bf[:, csl])

            # ---- phase 1: qkv projections ------------------------------
            # qkT[j, n] for j in [0, 1536): q rows 0-767, k rows 768-1535
            def emit_qk_group(jt: int, c4: int):
                ps = psum_sc.tile([P, 1024], F32, tag="sc")
                for ct in range(CT):
                    nc.tensor.matmul(
                        ps[:, 0:QW],
                        lhsT=w_qkv_sb[:, ct, jt * P : (jt + 1) * P],
                        rhs=xT[:, ct, c4 * QW : (c4 + 1) * QW],
                        start=(ct == 0),
                        stop=(ct == CT - 1),
                    )
                nc.vector.tensor_copy(
                    out=qkT[:, jt, c4 * QW : (c4 + 1) * QW], in_=ps[:, 0:QW]
                )

            # v natural layout: v[n, e] = sum_c x[n, c] w_qkv[c, 1536 + e]
            def emit_v_group(nt: int, eo: int, ew: int):
                ps = psum_sc.tile([P, 1024], F32, tag="sc")
                for ct in range(CT):
                    nc.tensor.matmul(
                        ps[:, 0:ew],
                        lhsT=xT[:, ct, nt * P : (nt + 1) * P],
                        rhs=w_qkv_sb[:, ct, 2 * C + eo : 2 * C + eo + ew],
                        start=(ct == 0),
                        stop=(ct == CT - 1),
                    )
                nc.vector.tensor_copy(out=v4[:, nt, eo : eo + ew], in_=ps[:, 0:ew])

            # upfront: only quad 0's needs — kT for pairs 0/1 and their
            # chunk-0 qT.  Everything else (later quads' kT/qT, v tiles,
            # later chunks' qT) is emitted just-in-time inside the attention
            # loops so ScalarE starts exping within ~15us of kernel start.
            for c4 in range(NCHUNK):
                emit_qk_group(6, c4)
            for c4 in range(NCHUNK):
                emit_qk_group(7, c4)
            emit_qk_group(0, 0)
            emit_qk_group(1, 0)
            # chunk-0 quad q prefetches quad q+1's kT (8 groups) + qT (2)
            c0_slots = {
                1: ("k", 0, 0), 2: ("k", 0, 1), 3: ("k", 0, 2), 4: ("k", 0, 3),
                5: ("k", 1, 0), 6: ("k", 1, 1), 7: ("k", 1, 2), 8: ("k", 1, 3),
                9: ("q", 0, 0), 10: ("q", 1, 0),
            }
            # chunk c's qT groups are emitted during chunk c-1, quad 2
            qt_slots = {2: 0, 5: 1, 8: 2, 11: 3, 13: 4, 15: 5}  # m -> jt

            # ---- phase 2+3: attention + projection ---------------------
            def emit_proj_group(nt: int, eo: int, ew: int):
                """final[nt-tile, eo:eo+ew] = outT^T w_proj + b."""
                ps = psum_proj.tile([P, 512], F32, tag="proj")
                for ct in range(CT):
                    nc.tensor.matmul(
                        ps[:, 0:ew],
                        lhsT=outT[:, ct, nt * P : (nt + 1) * P],
                        rhs=wproj_sb[:, ct, eo : eo + ew],
                        start=(ct == 0),
                        stop=(ct == CT - 1),
                    )
                fs = fin_pool.tile([P, 512], F32, tag="fin")
                nc.vector.tensor_tensor(
                    fs[:, 0:ew], ps[:, 0:ew], b_bc[:, eo : eo + ew],
                    mybir.AluOpType.add,
                )
                nc.sync.dma_start(
                    out=out[nt * P : (nt + 1) * P, eo : eo + ew], in_=fs[:, 0:ew]
                )

            # proj work for chunk c-1 is spread through chunk c's m-loops
            # (slots on quad 0/1 at fixed m) to avoid starving ScalarE.
            proj_slots = {  # (quad, m) -> slot index 0..7
                (0, 3): 0, (0, 7): 1, (0, 11): 2, (0, 14): 3,
                (1, 3): 4, (1, 7): 5, (1, 11): 6, (1, 14): 7,
            }

            def emit_proj_slot(c_done: int, slot: int):
                nt = c_done * 4 + slot // 2
                eo, ew = ((0, 512), (512, 256))[slot % 2]
                emit_proj_group(nt, eo, ew)

            for c in range(NCHUNK):
                qsl = slice(c * QW, (c + 1) * QW)
                for quad in range(3):
                    attn_ps = [
                        psum_av.tile([P, QW], F32, tag="av", name=f"av{pp}")
                        for pp in range(2)
                    ]
                    sums_ps = psum_sums.tile([P, QW], F32, tag="sums")
                    # only rows {0,32,64,96} get matmul results; init the rest
                    # so the full-tile reciprocal below reads defined memory
                    nc.vector.memset(sums_ps[:], 1.0)
                    for m in range(NT):
                        msl = slice(m * P, (m + 1) * P)
                        # just-in-time work: chunk 0 emits exactly the v
                        # columns this quad's attnV consumes, plus the next
                        # quad's kT/qT; quad 2 prefetches next chunk's qT.
                        if c == 0:
                            emit_v_group(m, quad * 256, 256)
                            if quad < 2 and m in c0_slots:
                                kind, i, c4s = c0_slots[m]
                                if kind == "k":
                                    emit_qk_group(8 + 2 * quad + i, c4s)
                                else:
                                    emit_qk_group(2 + 2 * quad + i, 0)
                        if quad == 2 and c < NCHUNK - 1 and m in qt_slots:
                            emit_qk_group(qt_slots[m], c + 1)
                        at_pair = []
                        for pp in range(2):
                            pair = 2 * quad + pp
                            sc = psum_sc.tile([P, 1024], F32, tag="sc")
                            # scoresT chunk: keys msl on partitions, queries
                            # qsl on free dim.  Head A rows 0-63, head B
                            # rows 64-127 -> auto row-tiled 64x128 pair.
                            nc.tensor.matmul(
                                sc[:, 0:QW],
                                lhsT=qkT[0:64, 6 + pair, msl],
                                rhs=qkT[0:64, pair, qsl],
                                start=True,
                                stop=True,
                            )
                            nc.tensor.matmul(
                                sc[:, QW : 2 * QW],
                                lhsT=qkT[64:128, 6 + pair, msl],
                                rhs=qkT[64:128, pair, qsl],
                                start=True,
                                stop=True,
                            )
                            at = at_pool.tile([P, 1024], BF16, tag="at")
                            nc.scalar.activation(at[:], sc[:], AF.Exp, scale=SCALE)
                            at_pair.append(at)
                        for pp in range(2):
                            pair = 2 * quad + pp
                            at = at_pair[pp]
                            for hh in range(2):
                                h = 2 * pair + hh
                                # attnV: col-tiled head pair, one PSUM bank
                                nc.tensor.matmul(
                                    attn_ps[pp][hh * 64 : (hh + 1) * 64, :],
                                    lhsT=v4[:, m, h * 64 : (h + 1) * 64],
                                    rhs=at[:, hh * QW : (hh + 1) * QW],
                                    start=(m == 0),
                                    stop=(m == NT - 1),
                                    # the sim's group-check view is partition-
                                    # blind; only the first col tile of the
                                    # shared bank may do the bookkeeping
                                    skip_group_check=(hh != 0),
                                )
                        for pp in range(2):
                            at = at_pair[pp]
                            for hh in range(2):
                                k4 = 2 * pp + hh
                                # denominators: 4 heads as 128x32 col tiles
                                nc.tensor.matmul(
                                    sums_ps[k4 * 32 : k4 * 32 + 1, :],
                                    lhsT=ones128[:, 0:1],
                                    rhs=at[:, hh * QW : (hh + 1) * QW],
                                    start=(m == 0),
                                    stop=(m == NT - 1),
                                    skip_group_check=(k4 != 0),
                                    tile_position=(0, k4 * 32),
                                )
                        if c > 0 and (quad, m) in proj_slots:
                            emit_proj_slot(c - 1, proj_slots[(quad, m)])

                    # ---- normalize + evict this (quad, chunk) ----------
                    recip_sb = recip_pool.tile([P, QW], F32, tag="recip")
                    nc.vector.reciprocal(recip_sb[:], sums_ps[:])
                    # bounce the 4 live rows through DRAM so a DMA can
                    # broadcast them across partitions
                    r_dram = rdpool.tile([4, QW], F32)
                    nc.sync.dma_start(out=r_dram[:], in_=recip_sb[0:97:32, :])
                    for pp in range(2):
                        rbc = rbc_pool.tile([P, QW], F32, tag="rbc")
                        nc.sync.dma_start(
                            out=rbc[0:64, :],
                            in_=r_dram[2 * pp : 2 * pp + 1, :].to_broadcast((64, QW)),
                        )
                        nc.sync.dma_start(
                            out=rbc[64:128, :],
                            in_=r_dram[2 * pp + 1 : 2 * pp + 2, :].to_broadcast(
                                (64, QW)
                            ),
                        )
                        nc.vector.tensor_tensor(
                            outT[:, 2 * quad + pp, qsl],
                            attn_ps[pp][:],
                            rbc[:],
                            mybir.AluOpType.mult,
                        )
            # tail: proj for the last chunk
            for slot in range(8):
                emit_proj_slot(NCHUNK - 1, slot)

    nc.compile()
    return nc


_NC_CACHE: list = []


def _get_nc() -> bass.Bass:
    if not _NC_CACHE:
        _NC_CACHE.append(build_nc())
    return _NC_CACHE[0]


def run(inputs: dict, trace: bool = False):
    """Run on 8 NeuronCores.  Returns (out [B,N,C] f32, exec_time_ns|None)."""
    nc = _get_nc()
    x = np.ascontiguousarray(np.asarray(inputs["x"], dtype=np.float32))
    w_qkv = np.ascontiguousarray(np.asarray(inputs["w_qkv"], dtype=np.float32))
    w_proj = np.ascontiguousarray(np.asarray(inputs["w_proj"], dtype=np.float32))
    b_proj = np.ascontiguousarray(np.asarray(inputs["b_proj"], dtype=np.float32))
    in_maps = [
        {"x": x[i], "w_qkv": w_qkv, "w_proj": w_proj, "b_proj": b_proj}
        for i in range(B)
    ]
    try:
        res = bass_utils.run_bass_kernel_spmd(
            nc, in_maps, core_ids=list(range(B)), trace=trace
        )
    except ModuleNotFoundError:
        # NTFF profile hook unavailable in this image; run without trace
        res = bass_utils.run_bass_kernel_spmd(
            nc, in_maps, core_ids=list(range(B)), trace=False
        )
    out = np.stack([res.results[i]["out"] for i in range(B)], axis=0)
    return out.astype(np.float32), res.exec_time_ns


def kernel(x, w_qkv, w_proj, b_proj):
    trace = os.environ.get("BASS_KERNEL_TRACE", "0") == "1"
    out, _ = run(
        {"x": x, "w_qkv": w_qkv, "w_proj": w_proj, "b_proj": b_proj}, trace=trace
    )
    return out



# revision 3
# speedup vs baseline: 1.7179x; 1.7179x over previous
"""Trainium2 Bass kernel v2 for multi-head attention (dense transformer).

Reference computation (per batch element):
    qkv = x @ w_qkv                      # [N, 3C]
    q, k, v = split heads (H=12, HD=64); q *= HD**-0.5
    out = softmax(q k^T) v               # full [N, N] scores
    out = merge_heads(out) @ w_proj + b_proj

Distribution: pure data parallel over batch — 8 elements, 8 cores.

Cost-model-driven redesign vs v1: the CoreSim cost model charges a matmul
out_free_dim x 0.42ns regardless of K/M, so the v1 denominator matmuls
(768 x FD=512) cost a full 166us and the col-packed attnV another 166us.
v2 restructures:
  * attnV is FLIPPED: lhsT = at-tile [128 keys, 128 queries] (stationary),
    rhs = v_ext [128 keys, 65] where column 64 is ones -> out [128 q, 65]
    accumulated over the 16 key tiles.  M=128 doubles the output per
    instruction (FD=65 vs 512), and the softmax denominator rides along as
    column 64 for +1 cycle.  attnV+denominators: 332us -> 83us.
  * normalization becomes a per-partition tensor_scalar DIVIDE on DVE
    (queries live on partitions), killing v1's DMA-bounce broadcast.
  * out is produced in [token, channel] layout; 96 PE transposes (~5us)
    restore the lhsT layout for the final projection.
  * sweep order is quad-major (3 sweeps of 4 heads x full N), so the
    qkT/v prefetch for quad g+1 hides inside quad g's ACT-bound sweep and
    the proj/transpose work hides inside quad 2's sweep.
ScalarE (384 x FD=1024 exp, ~399us busy) is the design floor; everything
else is scheduled to keep it saturated.

PSUM budget (8 banks): scores 2x[128,1024]f32 (4) + attnV accum 7/7/2
slots of [128,65]f32 (3) + one shared spare bank for qk/v/proj/transpose
groups (1).
"""

import os

import numpy as np

import concourse.bass as bass
import concourse.mybir as mybir
from concourse import bacc, bass_utils
from concourse.masks import make_identity
from concourse.tile import TileContext

F32 = mybir.dt.float32
BF16 = mybir.dt.bfloat16
AF = mybir.ActivationFunctionType
ALU = mybir.AluOpType

B, N, C = 8, 2048, 768
H, HD = 12, 64
SCALE = HD ** -0.5  # folded into the exp activation
P = 128
NT = N // P          # 16 key tiles
CT = C // P          # 6 feature tiles
NCH = 4              # query chunks per sweep
QW = N // NCH        # 512
QS = QW // P         # 4 query subtiles per chunk


def build_nc() -> bass.Bass:
    nc = bacc.Bacc(None)
    x = nc.declare_dram_parameter("x", [N, C], F32, isOutput=False)
    w_qkv = nc.declare_dram_parameter("w_qkv", [C, 3 * C], F32, isOutput=False)
    w_proj = nc.declare_dram_parameter("w_proj", [C, C], F32, isOutput=False)
    b_proj = nc.declare_dram_parameter("b_proj", [C], F32, isOutput=False)
    out = nc.declare_dram_parameter("out", [N, C], F32, isOutput=True)

    with TileContext(nc) as tc:
        with (
            tc.tile_pool(name="const", bufs=1) as cpool,
            tc.tile_pool(name="dram", bufs=1, space="DRAM") as dpool,
            tc.tile_pool(name="at", bufs=10) as at_pool,
            tc.tile_pool(name="rcp", bufs=2) as rcp_pool,
            tc.tile_pool(name="fin", bufs=2) as fin_pool,
            tc.tile_pool(name="psc", bufs=2, space="PSUM") as psum_sc,
            tc.tile_pool(name="pacc", bufs=1, space="PSUM") as psum_acc,
            tc.tile_pool(name="psp", bufs=1, space="PSUM") as psum_sp,
        ):
            # ---- persistent SBUF tensors -------------------------------
            w_qkv_sb = cpool.tile([P, CT, 3 * C], BF16, tag="wqkv")
            wproj_sb = cpool.tile([P, CT, C], BF16, tag="wproj")
            b_bc = cpool.tile([P, C], F32, tag="bias")
            xT = cpool.tile([P, CT, N], BF16, tag="xT")
            qkT = cpool.tile([P, 12, N], BF16, tag="qkT")  # q pairs 0-5, k 6-11
            v4 = cpool.tile([P, NT, H, HD + 1], BF16, tag="v4")  # col 64 = ones
            out_sb = cpool.tile([P, NT, C], BF16, tag="outsb")  # [token, chan]
            outT = cpool.tile([P, CT, N], BF16, tag="outT")
            ident = cpool.tile([P, P], BF16, tag="ident")
            warm = cpool.tile([P, 1], F32, tag="warm")

            # ---- phase 0: act-table preload, consts, input DMAs --------
            nc.vector.memset(warm[:], 0.0)
            nc.scalar.activation(warm[:], warm[:], AF.Exp)  # preload exp table
            make_identity(nc, ident)
            nc.vector.memset(v4[:, :, :, HD], 1.0)  # denominator ones columns

            x_bf = dpool.tile([N, C], BF16)
            wq_r = w_qkv.rearrange("(o p) j -> p o j", p=P)
            for ct in range(CT):
                csl = slice(ct * P, (ct + 1) * P)
                nc.gpsimd.dma_start(out=x_bf[:, csl], in_=x[:, csl])
                # split the transpose chain across the two HWDGE queues
                eng = nc.sync if ct % 2 == 0 else nc.scalar
                eng.dma_start_transpose(xT[:, ct, :], x_bf[:, csl])
            for ct in range(CT):
                nc.gpsimd.dma_start(out=w_qkv_sb[:, ct, :], in_=wq_r[:, ct, :])
            nc.sync.dma_start(out=b_bc[:], in_=b_proj[None, :].to_broadcast((P, C)))
            nc.gpsimd.dma_start(
                out=wproj_sb[:], in_=w_proj.rearrange("(o p) j -> p o j", p=P)
            )

            # ---- emit helpers ------------------------------------------
            def emit_qk_group(jt: int, c4: int, pool, tag: str):
                """qkT[:, jt, c4*512:...] = w_qk(jt)^T @ x^T chunk."""
                ps = pool.tile([P, QW], F32, tag=tag, name=f"qk{jt}_{c4}")
                for ct in range(CT):
                    nc.tensor.matmul(
                        ps[:, 0:QW],
                        lhsT=w_qkv_sb[:, ct, jt * P:(jt + 1) * P],
                        rhs=xT[:, ct, c4 * QW:(c4 + 1) * QW],
                        start=(ct == 0),
                        stop=(ct == CT - 1),
                    )
                nc.vector.tensor_copy(
                    out=qkT[:, jt, c4 * QW:(c4 + 1) * QW], in_=ps[:, 0:QW]
                )

            def emit_v_group(m: int, g: int):
                """v for key tile m, heads 4g..4g+3, natural [key, hd] layout."""
                ps = psum_sp.tile([P, QW], F32, tag="sp", name=f"v{g}_{m}")
                for ct in range(CT):
                    nc.tensor.matmul(
                        ps[:, 0:256],
                        lhsT=xT[:, ct, m * P:(m + 1) * P],
                        rhs=w_qkv_sb[:, ct, 2 * C + g * 256: 2 * C + (g + 1) * 256],
                        start=(ct == 0),
                        stop=(ct == CT - 1),
                    )
                nc.vector.tensor_copy(
                    out=v4[:, m, 4 * g:4 * (g + 1), 0:HD], in_=ps[:, 0:256]
                )

            def emit_transpose(nt_: int):
                """outT[:, :, nt] = out_sb[nt]^T via 6 PE-mode transposes."""
                tp = psum_sp.tile([P, 8, P], BF16, tag="sp", name=f"tp{nt_}")
                for ct in range(CT):
                    nc.tensor.transpose(
                        tp[:, ct, :], out_sb[:, nt_, ct * P:(ct + 1) * P], ident
                    )
                nsl = slice(nt_ * P, (nt_ + 1) * P)
                nc.vector.tensor_copy(out=outT[:, 0:4, nsl], in_=tp[:, 0:4, :])
                nc.vector.tensor_copy(out=outT[:, 4:6, nsl], in_=tp[:, 4:6, :])

            def emit_proj(nt_: int, slot_: int, pool, tag: str):
                """final[nt tile, eo:eo+ew] = outT^T @ w_proj + b."""
                eo, ew = ((0, QW), (QW, 256))[slot_]
                ps = pool.tile([P, QW], F32, tag=tag, name=f"pj{nt_}_{slot_}")
                for ct in range(CT):
                    nc.tensor.matmul(
                        ps[:, 0:ew],
                        lhsT=outT[:, ct, nt_ * P:(nt_ + 1) * P],
                        rhs=wproj_sb[:, ct, eo:eo + ew],
                        start=(ct == 0),
                        stop=(ct == CT - 1),
                    )
                fs = fin_pool.tile([P, QW], F32, tag="fin")
                nc.vector.tensor_tensor(
                    fs[:, 0:ew], ps[:, 0:ew], b_bc[:, eo:eo + ew], ALU.add
                )
                nc.sync.dma_start(
                    out=out[nt_ * P:(nt_ + 1) * P, eo:eo + ew], in_=fs[:, 0:ew]
                )

            # ---- spare-bank task plan per (quad, chunk) window ---------
            def QK(jt, c4):
                return lambda: emit_qk_group(jt, c4, psum_sp, "sp")

            def VG(g, m):
                return lambda: emit_v_group(m, g)

            def TR(nt_):
                return lambda: emit_transpose(nt_)

            def PJ(nt_, s_):
                return lambda: emit_proj(nt_, s_, psum_sp, "sp")

            # plan[(q, c)][m] = spare tasks emitted at iteration (q, c, m),
            # placed BEFORE that iteration's attnV and AFTER its lookahead
            # scores.  Deadlines: VG(g, m') by iteration m' of g's first
            # chunk; kT QK(jt, c4) one iteration before scores m=4*c4; next
            # chunk's qT one iteration before the window ends.
            plan = {
                # quad 0 sweep: own kT/qT tails + v heads 0-3; prefetch quad 1
                (0, 0): {0: [VG(0, 0)], 1: [VG(0, 1), QK(6, 1), QK(7, 1)],
                         2: [VG(0, 2)], 3: [VG(0, 3)],
                         4: [VG(0, 4), QK(6, 2)], 5: [VG(0, 5), QK(7, 2)],
                         6: [VG(0, 6)], 7: [VG(0, 7), QK(6, 3)],
                         8: [VG(0, 8)], 9: [VG(0, 9), QK(7, 3)],
                         10: [VG(0, 10)], 11: [VG(0, 11), QK(0, 1)],
                         12: [VG(0, 12)], 13: [VG(0, 13), QK(1, 1)],
                         14: [VG(0, 14)], 15: [VG(0, 15)]},
                (0, 1): {0: [QK(0, 2)], 2: [QK(1, 2)], 4: [QK(8, 0)],
                         6: [QK(9, 0)]},
                (0, 2): {0: [QK(0, 3)], 2: [QK(1, 3)], 4: [QK(8, 1)],
                         6: [QK(9, 1)]},
                (0, 3): {0: [QK(8, 2)], 2: [QK(9, 2)], 4: [QK(2, 0)],
                         6: [QK(3, 0)], 8: [VG(1, 0)], 10: [VG(1, 1)],
                         12: [VG(1, 2)], 14: [VG(1, 3)]},
                # quad 1 sweep
                (1, 0): {0: [QK(8, 3)], 1: [QK(9, 3)], 2: [VG(1, 4)],
                         3: [VG(1, 5)], 4: [VG(1, 6)], 5: [VG(1, 7)],
                         6: [VG(1, 8)], 7: [VG(1, 9)], 8: [VG(1, 10)],
                         9: [VG(1, 11)], 10: [VG(1, 12)], 11: [VG(1, 13)],
                         12: [VG(1, 14), QK(2, 1)], 13: [VG(1, 15), QK(3, 1)]},
                (1, 1): {0: [QK(2, 2)], 2: [QK(3, 2)], 4: [QK(10, 0)],
                         6: [QK(11, 0)]},
                (1, 2): {0: [QK(2, 3)], 2: [QK(3, 3)], 4: [QK(10, 1)],
                         6: [QK(11, 1)]},
                (1, 3): {0: [QK(10, 2)], 2: [QK(11, 2)], 4: [QK(4, 0)],
                         6: [QK(5, 0)], 8: [VG(2, 0)], 10: [VG(2, 1)],
                         12: [VG(2, 2)], 14: [VG(2, 3)]},
                # quad 2 sweep: remaining prefetch, then transposes + proj
                (2, 0): {0: [QK(10, 3)], 1: [QK(11, 3)], 2: [VG(2, 4)],
                         3: [VG(2, 5)], 4: [VG(2, 6)], 5: [VG(2, 7)],
                         6: [VG(2, 8)], 7: [VG(2, 9)], 8: [VG(2, 10)],
                         9: [VG(2, 11)], 10: [VG(2, 12)], 11: [VG(2, 13)],
                         12: [VG(2, 14), QK(4, 1)], 13: [VG(2, 15), QK(5, 1)]},
                (2, 1): {0: [QK(4, 2)], 1: [QK(5, 2)], 2: [TR(0)],
                         3: [PJ(0, 0)], 4: [PJ(0, 1)], 5: [TR(1)],
                         6: [PJ(1, 0)], 7: [PJ(1, 1)], 8: [TR(2)],
                         9: [PJ(2, 0)], 10: [PJ(2, 1)], 11: [TR(3)],
                         12: [PJ(3, 0)], 13: [PJ(3, 1)]},
                (2, 2): {0: [QK(4, 3)], 1: [QK(5, 3)], 2: [TR(4)],
                         3: [PJ(4, 0)], 4: [PJ(4, 1)], 5: [TR(5)],
                         6: [PJ(5, 0)], 7: [PJ(5, 1)], 8: [TR(6)],
                         9: [PJ(6, 0)], 10: [PJ(6, 1)], 11: [TR(7)],
                         12: [PJ(7, 0)], 13: [PJ(7, 1)]},
                (2, 3): {0: [TR(8)], 1: [PJ(8, 0)], 2: [PJ(8, 1)],
                         3: [TR(9)], 4: [PJ(9, 0)], 5: [PJ(9, 1)],
                         6: [TR(10)], 7: [PJ(10, 0)], 8: [PJ(10, 1)],
                         9: [TR(11)], 10: [PJ(11, 0)], 11: [PJ(11, 1)]},
            }

            # ---- attention sweep machinery -----------------------------
            acc_tiles: dict = {}

            def slot_ap(gen, slot):
                a, b_, c_ = acc_tiles[gen]
                if slot < 7:
                    return a, slot
                if slot < 14:
                    return b_, slot - 7
                return c_, slot - 14

            at_hist: dict = {}

            def emit_scores_exp(q: int, c: int, m: int):
                qsl = slice(c * QW, (c + 1) * QW)
                msl = slice(m * P, (m + 1) * P)
                ats = []
                for pp in range(2):
                    pair = 2 * q + pp
                    sc = psum_sc.tile(
                        [P, 1024], F32, tag="sc", name=f"sc{q}_{c}_{m}_{pp}"
                    )
                    # scoresT: keys msl on partitions, queries qsl on free dim;
                    # head 2p on PE rows 0-63, head 2p+1 on rows 64-127
                    nc.tensor.matmul(
                        sc[:, 0:QW],
                        lhsT=qkT[0:64, 6 + pair, msl],
                        rhs=qkT[0:64, pair, qsl],
                        start=True,
                        stop=True,
                    )
                    nc.tensor.matmul(
                        sc[:, QW:2 * QW],
                        lhsT=qkT[64:128, 6 + pair, msl],
                        rhs=qkT[64:128, pair, qsl],
                        start=True,
                        stop=True,
                    )
                    at = at_pool.tile(
                        [P, 1024], BF16, tag="at", name=f"at{q}_{c}_{m}_{pp}"
                    )
                    nc.scalar.activation(at[:], sc[:], AF.Exp, scale=SCALE)
                    ats.append(at)
                at_hist[(q, c, m)] = ats

            def emit_attnv(q: int, c: int, m: int):
                gen = (q, c)
                if m == 0:
                    acc_tiles[gen] = (
                        psum_acc.tile([P, 7, HD + 1], F32, tag="acca",
                                      name=f"acca{q}_{c}"),
                        psum_acc.tile([P, 7, HD + 1], F32, tag="accb",
                                      name=f"accb{q}_{c}"),
                        psum_acc.tile([P, 2, HD + 1], F32, tag="accc",
                                      name=f"accc{q}_{c}"),
                    )
                ats = at_hist.pop((q, c, m))
                for pp in range(2):
                    for hh in range(2):
                        for s in range(QS):
                            slot = pp * 8 + hh * 4 + s
                            tile_, idx = slot_ap(gen, slot)
                            # flipped attnV: out[q, hd+1] = at^T @ [v | 1].
                            # start=True clears has_written for the WHOLE
                            # bank, so only the first slot per bank (0/7/14)
                            # may set it; later slots' m=0 writes land on
                            # bank-cleared bytes and overwrite implicitly.
                            nc.tensor.matmul(
                                tile_[:, idx, :],
                                lhsT=ats[pp][:, hh * QW + s * P:
                                             hh * QW + (s + 1) * P],
                                rhs=v4[:, m, 4 * q + 2 * pp + hh, :],
                                start=(m == 0 and slot in (0, 7, 14)),
                                stop=(m == NT - 1),
                                skip_group_check=(slot not in (0, 7, 14)),
                            )

            def emit_normalize(q: int, c: int):
                gen = (q, c)
                a, b_, c_ = acc_tiles[gen]
                # reciprocal of the 16 denominators (column 64 of each slot);
                # HW TensorScalarPtr has no divide ALU op
                rc = rcp_pool.tile([P, 16], F32, tag="rcp", name=f"rc{q}_{c}")
                nc.vector.reciprocal(rc[:, 0:7], a[:, :, HD])
                nc.vector.reciprocal(rc[:, 7:14], b_[:, :, HD])
                nc.vector.reciprocal(rc[:, 14:16], c_[:, :, HD])
                for slot in range(16):
                    pp, hh, s = slot // 8, (slot // 4) % 2, slot % 4
                    head = 4 * q + 2 * pp + hh
                    nt_ = c * QS + s
                    tile_, idx = slot_ap(gen, slot)
                    nc.vector.tensor_scalar(
                        out=out_sb[:, nt_, head * HD:(head + 1) * HD],
                        in0=tile_[:, idx, 0:HD],
                        scalar1=rc[:, slot:slot + 1],
                        scalar2=None,
                        op0=ALU.mult,
                    )
                del acc_tiles[gen]

            # ---- main loop: software-pipelined quad-major sweeps -------
            # upfront qkT groups on the scores psum ring (attention idle)
            emit_qk_group(6, 0, psum_sc, "sc")
            emit_qk_group(0, 0, psum_sc, "sc")
            emit_qk_group(7, 0, psum_sc, "sc")
            emit_qk_group(1, 0, psum_sc, "sc")

            iters = [(q, c, m) for q in range(3) for c in range(NCH)
                     for m in range(NT)]
            emit_scores_exp(*iters[0])
            for i, (q, c, m) in enumerate(iters):
                # issue next iteration's scores first so ScalarE never waits
                if i + 1 < len(iters):
                    emit_scores_exp(*iters[i + 1])
                # spare tasks precede attnV: v-group m must be defined before
                # the attnV that consumes it (program order = PE queue order)
                for t in plan.get((q, c), {}).get(m, ()):
                    t()
                emit_attnv(q, c, m)
                if m == NT - 1:
                    emit_normalize(q, c)

            # ---- tail: last chunk's transposes + projections -----------
            for nt_ in range(12, 16):
                emit_transpose(nt_)
            for nt_ in range(12, 16):
                for s_ in range(2):
                    emit_proj(nt_, s_, psum_sc, "sc")

            _DBG_TILES.update(
                qkT=qkT, v4=v4, out_sb=out_sb, outT=outT, xT=xT,
                w_qkv_sb=w_qkv_sb,
            )

    nc.compile()
    return nc


_DBG_TILES: dict = {}


_NC_CACHE: list = []


def _get_nc() -> bass.Bass:
    if not _NC_CACHE:
        _NC_CACHE.append(build_nc())
    return _NC_CACHE[0]


def run(inputs: dict, trace: bool = False):
    """Run on 8 NeuronCores.  Returns (out [B,N,C] f32, exec_time_ns|None)."""
    nc = _get_nc()
    x = np.ascontiguousarray(np.asarray(inputs["x"], dtype=np.float32))
    w_qkv = np.ascontiguousarray(np.asarray(inputs["w_qkv"], dtype=np.float32))
    w_proj = np.ascontiguousarray(np.asarray(inputs["w_proj"], dtype=np.float32))
    b_proj = np.ascontiguousarray(np.asarray(inputs["b_proj"], dtype=np.float32))
    in_maps = [
        {"x": x[i], "w_qkv": w_qkv, "w_proj": w_proj, "b_proj": b_proj}
        for i in range(B)
    ]
    try:
        res = bass_utils.run_bass_kernel_spmd(
            nc, in_maps, core_ids=list(range(B)), trace=trace
        )
    except ModuleNotFoundError:
        res = bass_utils.run_bass_kernel_spmd(
            nc, in_maps, core_ids=list(range(B)), trace=False
        )
    out = np.stack([res.results[i]["out"] for i in range(B)], axis=0)
    return out.astype(np.float32), res.exec_time_ns


def kernel(x, w_qkv, w_proj, b_proj):
    trace = os.environ.get("BASS_KERNEL_TRACE", "0") == "1"
    out, _ = run(
        {"x": x, "w_qkv": w_qkv, "w_proj": w_proj, "b_proj": b_proj}, trace=trace
    )
    return out
